# revision 1
# baseline (speedup 1.0000x reference)
"""Causal self-attention with RoPE for trn2, 8-core SPMD.

Sharding: core i handles batch b = i//2 and heads [8*(i%2), 8*(i%2)+8).
Each core computes a partial output [T, C] = y_local @ Wo_rows_local;
host sums core pairs and adds bo.

Layout strategy (per core, all matmuls in float32r):
  P1: qT/kT per head in [d, t] layout via lhsT=Wq tiles, rhs=xT tiles;
      bias added on ACT eviction; RoPE on DVE in even/odd-deinterleaved
      row order (weights column-permuted on host); v in [t, d] layout.
      q/k/v round-trip through internal DRAM.
  P2: per head flash-style: St[tk,tq] = kT_blk.T @ qT (K=d=128, single
      matmul), E = exp(scale*St) on ACT, binary-mask diag blocks on DVE,
      denom accumulated via ones-matmul (M=1), yT accumulated via
      lhsT=v_blk. Normalization folded into PSUM eviction using
      reciprocal + partition-broadcast DMA. No max-subtraction (scores
      are O(10), fp32 exp cannot overflow).
  P3: out[t, c] = sum_h yT_h[:, t-tile].T @ Wo rows.
"""
from contextlib import ExitStack

import numpy as np

import concourse.bacc as bacc
import concourse.tile as tile
from concourse import mybir

F32 = mybir.dt.float32
F32R = mybir.dt.float32r
AF = mybir.ActivationFunctionType
ALU = mybir.AluOpType

N_HEAD = 16
HEAD_DIM = 128
ROPE_BASE = 10000.0


def r32(ap):
    return ap


def build_core_kernel(T=2048, C=2048, HL=8, reps=1, pool_mode="stack"):
    """One core's program: full T, C channels, HL local heads."""
    D = HEAD_DIM
    CL = HL * D            # local q/k/v channels (1024)
    NCT = C // 128         # c-tiles (16)
    NQ = T // 512          # 512-wide t slices (4)
    NT = T // 128          # 128-wide t tiles (16)
    NG = CL // 512         # v column groups (2)
    NCQ = C // 512         # out-proj column groups (4)
    scale = 1.0 / float(np.sqrt(D))

    nc = bacc.Bacc("TRN2", target_bir_lowering=False, debug=False)

    xT_d = nc.dram_tensor("xT", [C, T], F32R, kind="ExternalInput")
    wq_d = nc.dram_tensor("wq", [HL, 128, NCT, 128], F32R, kind="ExternalInput")
    wk_d = nc.dram_tensor("wk", [HL, 128, NCT, 128], F32R, kind="ExternalInput")
    wv_d = nc.dram_tensor("wv", [NG, 128, NCT, 512], F32R, kind="ExternalInput")
    wo_d = nc.dram_tensor("wo", [128, HL, NCQ, 512], F32R, kind="ExternalInput")
    bq_d = nc.dram_tensor("bq", [CL], F32, kind="ExternalInput")
    bk_d = nc.dram_tensor("bk", [CL], F32, kind="ExternalInput")
    bv_d = nc.dram_tensor("bv", [CL], F32, kind="ExternalInput")
    cos_d = nc.dram_tensor("cos2", [128, T], F32, kind="ExternalInput")
    sin_d = nc.dram_tensor("sin2", [128, T], F32, kind="ExternalInput")
    mask_d = nc.dram_tensor("masks", [4, 128, 512], F32, kind="ExternalInput")
    ones_d = nc.dram_tensor("ones", [128, 128], F32R, kind="ExternalInput")
    out_d = nc.dram_tensor("out_p", [T, C], F32, kind="ExternalOutput")

    with tile.TileContext(nc, pool_alloc_mode=pool_mode) as tc, ExitStack() as top:
        dram = top.enter_context(tc.tile_pool(name="dram", bufs=1, space="DRAM"))
        q_rt = dram.tile([HL, 128, T], F32R)
        k_rt = dram.tile([HL, 128, T], F32R)
        v_rt = dram.tile([NT, NG, 128, 512], F32R)
        y_rt = dram.tile([HL, 128, T], F32R)

        psA = top.enter_context(tc.tile_pool(name="psA", bufs=4, space="PSUM"))
        psY = top.enter_context(tc.tile_pool(name="psY", bufs=2, space="PSUM"))
        psD = top.enter_context(tc.tile_pool(name="psD", bufs=2, space="PSUM"))

        const = top.enter_context(tc.tile_pool(name="const", bufs=1))
        ones_sb = const.tile([128, 128], F32R)
        nc.sync.dma_start(out=ones_sb, in_=ones_d[:, :])

        rep_ctx = tc.For_i(0, reps, 1) if reps > 1 else None
        if rep_ctx is not None:
            top.enter_context(rep_ctx)

        # ---------------- P1: projections + rope ----------------
        # xT resident (128KB/part); weights loaded once per head/group.
        with ExitStack() as p1x:
            xp = p1x.enter_context(tc.tile_pool(name="xp", bufs=1))
            xT_sb = xp.tile([128, NCT, T], F32R)
            for ct in range(NCT):
                nc.sync.dma_start(
                    out=xT_sb[:, ct, :],
                    in_=xT_d[ct * 128 : (ct + 1) * 128, :],
                )

            # v first (frees wv pool before qk weights arrive)
            with ExitStack() as p1v:
                wvp = p1v.enter_context(tc.tile_pool(name="wvp", bufs=1))
                bvp = p1v.enter_context(tc.tile_pool(name="bvp", bufs=1))
                ev0 = p1v.enter_context(tc.tile_pool(name="ev0", bufs=3))
                bv_sb = bvp.tile([128, CL], F32)
                nc.sync.dma_start(out=bv_sb, in_=bv_d[:].partition_broadcast(128))
                for g in range(NG):
                    gs = slice(g * 512, (g + 1) * 512)
                    wv_sb = wvp.tile([128, NCT, 512], F32R)
                    nc.sync.dma_start(out=wv_sb, in_=wv_d[g])
                    for tt in range(NT):
                        xl = xT_sb[:, :, tt * 128 : (tt + 1) * 128]
                        ps = psA.tile([128, 512], F32, tag="mm")
                        for ct in range(NCT):
                            nc.tensor.matmul(
                                ps[:],
                                xl[:, ct, :],
                                wv_sb[:, ct, :],
                                start=(ct == 0),
                                stop=(ct == NCT - 1),
                            )
                        vt = ev0.tile([128, 512], F32R, tag="vt")
                        nc.vector.tensor_tensor(vt[:], ps[:], bv_sb[:, gs], op=ALU.add)
                        nc.sync.dma_start(out=v_rt[tt, g], in_=vt[:])

            # q/k per head, all four t-slices per weight load
            with ExitStack() as p1qk:
                wqk = p1qk.enter_context(tc.tile_pool(name="wqk", bufs=2))
                ev1 = p1qk.enter_context(tc.tile_pool(name="ev1", bufs=2))
                trig = p1qk.enter_context(tc.tile_pool(name="trig", bufs=1))
                cos_sb = trig.tile([128, T], F32)
                sin_sb = trig.tile([128, T], F32)
                nc.sync.dma_start(out=cos_sb, in_=cos_d[:, :])
                nc.sync.dma_start(out=sin_sb, in_=sin_d[:, :])
                bq_sb = trig.tile([128, HL], F32)
                bk_sb = trig.tile([128, HL], F32)
                nc.sync.dma_start(out=bq_sb, in_=bq_d.rearrange("(h p) -> p h", p=128))
                nc.sync.dma_start(out=bk_sb, in_=bk_d.rearrange("(h p) -> p h", p=128))

                for h in range(HL):
                    for w_d, b_sb, o_rt, wtag in (
                        (wq_d, bq_sb, q_rt, "wqs"),
                        (wk_d, bk_sb, k_rt, "wks"),
                    ):
                        w_sb = wqk.tile([128, NCT, 128], F32R, tag=wtag)
                        nc.sync.dma_start(out=w_sb, in_=w_d[h])
                        for s in range(NQ):
                            ts = slice(s * 512, (s + 1) * 512)
                            ps = psA.tile([128, 512], F32, tag="mm")
                            for ct in range(NCT):
                                nc.tensor.matmul(
                                    ps[:],
                                    w_sb[:, ct, :],
                                    xT_sb[:, ct, ts],
                                    start=(ct == 0),
                                    stop=(ct == NCT - 1),
                                )
                            raw = ev1.tile([128, 512], F32, tag="qkraw")
                            nc.scalar.activation(
                                out=raw[:], in_=ps[:], func=AF.Identity,
                                bias=b_sb[:, h : h + 1], scale=1.0,
                            )
                            p1t = ev1.tile([128, 512], F32, tag="p1t")
                            p2t = ev1.tile([128, 512], F32, tag="p2t")
                            nc.vector.tensor_tensor(p1t[:], raw[:], cos_sb[:, ts], op=ALU.mult)
                            nc.vector.tensor_tensor(p2t[:], raw[:], sin_sb[:, ts], op=ALU.mult)
                            swp = ev1.tile([128, 512], F32, tag="swp")
                            nc.sync.dma_start(out=swp[0:64, :], in_=p2t[64:128, :])
                            nc.sync.dma_start(out=swp[64:128, :], in_=p2t[0:64, :])
                            rot = ev1.tile([128, 512], F32R, tag="rot")
                            nc.vector.tensor_tensor(rot[:], p1t[:], swp[:], op=ALU.add)
                            nc.sync.dma_start(out=o_rt[h, :, ts], in_=rot[:])

        # ---------------- P2: attention per head ----------------
        wop_ctx = ExitStack()
        wop = wop_ctx.enter_context(tc.tile_pool(name="wop", bufs=1))
        wo_sb = wop.tile([128, HL, NCQ, 512], F32R)
        nc.sync.dma_start(out=wo_sb, in_=wo_d[:, :, :, :])
        with ExitStack() as p2:
            qkp = p2.enter_context(tc.tile_pool(name="qkp", bufs=2))
            vhp = p2.enter_context(tc.tile_pool(name="vhp", bufs=2))
            ep = p2.enter_context(tc.tile_pool(name="ep", bufs=6))
            yp = p2.enter_context(tc.tile_pool(name="yp", bufs=3))
            rp = p2.enter_context(tc.tile_pool(name="rp", bufs=3))
            mp = p2.enter_context(tc.tile_pool(name="mp", bufs=1))

            masks_sb = mp.tile([128, 4, 512], F32)
            nc.sync.dma_start(out=masks_sb, in_=mask_d.rearrange("m p f -> p m f"))

            for h in range(HL):
                g, off = h // 4, (h % 4) * 128
                q_sb = qkp.tile([128, T], F32R, tag="qh")
                k_sb = qkp.tile([128, T], F32R, tag="kh")
                v_sb = vhp.tile([128, NT, 128], F32R)
                nc.sync.dma_start(out=q_sb, in_=q_rt[h])
                nc.sync.dma_start(out=k_sb, in_=k_rt[h])
                nc.sync.dma_start(
                    out=v_sb,
                    in_=v_rt[:, g, :, off : off + 128].rearrange("n p d -> p n d"),
                )
                for j in range(NQ):
                    js = slice(j * 512, (j + 1) * 512)
                    nblk = 4 * (j + 1)
                    psd = psD.tile([128, 512], F32)
                    psy = psY.tile([128, 512], F32)
                    # software-pipelined: St/E one block ahead of den/y
                    etiles = []
                    for b in range(nblk):
                        pss = psA.tile([128, 512], F32, tag="mm")
                        nc.tensor.matmul(
                            pss[:],
                            r32(k_sb[:, b * 128 : (b + 1) * 128]),
                            r32(q_sb[:, js]),
                            start=True,
                            stop=True,
                        )
                        et = ep.tile([128, 512], F32R)
                        nc.scalar.activation(
                            out=et[:], in_=pss[:], func=AF.Exp, scale=scale
                        )
                        if b >= 4 * j:
                            nc.vector.tensor_tensor(
                                et[:], et[:], masks_sb[:, b - 4 * j, :], op=ALU.mult
                            )
                        etiles.append(et)
                        if b >= 1:
                            eprev = etiles[b - 1]
                            nc.tensor.matmul(
                                psd[:], r32(ones_sb[:]), r32(eprev[:]),
                                start=(b == 1), stop=False,
                            )
                            nc.tensor.matmul(
                                psy[:], r32(v_sb[:, b - 1, :]), r32(eprev[:]),
                                start=(b == 1), stop=False,
                            )
                    elast = etiles[nblk - 1]
                    nc.tensor.matmul(
                        psd[:], r32(ones_sb[:]), r32(elast[:]),
                        start=(nblk == 1), stop=True,
                    )
                    nc.tensor.matmul(
                        psy[:], r32(v_sb[:, nblk - 1, :]), r32(elast[:]),
                        start=(nblk == 1), stop=True,
                    )
                    recb = rp.tile([128, 512], F32, tag="recb")
                    nc.vector.reciprocal(out=recb[:], in_=psd[:])
                    yt = yp.tile([128, 512], F32R)
                    nc.vector.tensor_tensor(yt[:], psy[:], recb[:], op=ALU.mult)
                    nc.sync.dma_start(out=y_rt[h, :, js], in_=yt[:])

        # ---------------- P3: output projection ----------------
        with ExitStack() as p3:
            y3p = p3.enter_context(tc.tile_pool(name="y3p", bufs=3))
            op = p3.enter_context(tc.tile_pool(name="op", bufs=3))

            for tt in range(NT):
                tsl = slice(tt * 128, (tt + 1) * 128)
                yts = y3p.tile([128, HL, 128], F32R)
                nc.sync.dma_start(
                    out=yts, in_=y_rt[:, :, tsl].rearrange("h p t -> p h t")
                )
                for cq in range(NCQ):
                    ps = psA.tile([128, 512], F32, tag="mm")
                    for h in range(HL):
                        nc.tensor.matmul(
                            ps[:],
                            r32(yts[:, h, :]),
                            r32(wo_sb[:, h, cq, :]),
                            start=(h == 0),
                            stop=(h == HL - 1),
                        )
                    ot = op.tile([128, 512], F32)
                    nc.scalar.copy(out=ot[:], in_=ps[:])
                    nc.sync.dma_start(
                        out=out_d[tsl, cq * 512 : (cq + 1) * 512], in_=ot[:]
                    )
        wop_ctx.close()

    nc.finalize()
    return nc


def _col_perm(CL):
    """Per-head even/odd de-interleave of columns."""
    perm = []
    for h in range(CL // 128):
        base = h * 128
        perm += [base + i for i in range(0, 128, 2)]
        perm += [base + i for i in range(1, 128, 2)]
    return np.array(perm)


def host_prepare(x, Wq, bq, Wk, bk, Wv, bv, Wo, bo, T=None):
    """Build the 8 per-core input maps. x: [B, T, C] fp32."""
    B, Tfull, C = x.shape
    if T is None:
        T = Tfull
    D = HEAD_DIM
    perm = _col_perm(C)
    Wq_p = np.ascontiguousarray(Wq[:, perm])
    Wk_p = np.ascontiguousarray(Wk[:, perm])
    bq_p = np.ascontiguousarray(bq[perm])
    bk_p = np.ascontiguousarray(bk[perm])

    # rope tables, fp32 to mirror the reference computation
    inv = (1.0 / (ROPE_BASE ** (np.arange(0, D, 2, dtype=np.float32) / D))).astype(
        np.float32
    )
    pos = np.arange(T, dtype=np.float32)
    th = pos[None, :] * inv[:, None]          # [64, T]
    cos1 = np.cos(th).astype(np.float32)
    sin1 = np.sin(th).astype(np.float32)
    cos2 = np.concatenate([cos1, cos1], axis=0)
    sin2 = np.concatenate([sin1, -sin1], axis=0)

    m = np.zeros((4, 128, 512), dtype=np.float32)
    p = np.arange(128)[:, None]
    f = np.arange(512)[None, :]
    for mi in range(4):
        m[mi] = ((p + mi * 128) <= f).astype(np.float32)

    NCT = C // 128

    def tile_qk(W):  # [C, 1024] -> [8, 128, NCT, 128]
        return np.ascontiguousarray(
            W.reshape(NCT, 128, 8, 128).transpose(2, 1, 0, 3)
        )

    def tile_v(W):  # [C, 1024] -> [2, 128, NCT, 512]
        return np.ascontiguousarray(
            W.reshape(NCT, 128, 2, 512).transpose(2, 1, 0, 3)
        )

    def tile_o(W):  # [1024, C] -> [128, 8, C//512, 512]
        return np.ascontiguousarray(
            W.reshape(8, 128, C // 512, 512).transpose(1, 0, 2, 3)
        )

    in_maps = []
    for core in range(8):
        b, half = core // 2, core % 2
        cl = slice(half * 1024, (half + 1) * 1024)
        xT = np.ascontiguousarray(x[b, :T].T)
        in_maps.append(
            {
                "xT": xT,
                "wq": tile_qk(Wq_p[:, cl]),
                "wk": tile_qk(Wk_p[:, cl]),
                "wv": tile_v(Wv[:, cl]),
                "wo": tile_o(Wo[cl.start : cl.stop, :]),
                "bq": np.ascontiguousarray(bq_p[cl]),
                "bk": np.ascontiguousarray(bk_p[cl]),
                "bv": np.ascontiguousarray(bv[cl]),
                "cos2": cos2,
                "sin2": sin2,
                "masks": m,
                "ones": np.ones((128, 128), dtype=np.float32),
            }
        )
    return in_maps


def assemble(results, bo, B, T, C):
    out = np.empty((B, T, C), dtype=np.float32)
    for b in range(B):
        out[b] = results[2 * b]["out_p"] + results[2 * b + 1]["out_p"] + bo[None, :]
    return out


# ---------------------------------------------------------------------------
# SPMD execution via PJRT/axon (compiles once per process, reusable)
# ---------------------------------------------------------------------------
import jax
from jax.sharding import Mesh, PartitionSpec
from jax.experimental.shard_map import shard_map

from concourse.bass2jax import (
    _bass_exec_p,
    install_neuronx_cc_hook,
    partition_id_tensor,
)


class _SpmdRunner:
    def __init__(self, nc, n_cores):
        install_neuronx_cc_hook()
        self.nc = nc
        self.n_cores = n_cores
        partition_name = (
            nc.partition_id_tensor.name if nc.partition_id_tensor else None
        )
        in_names, out_names, out_avals, zero_outs = [], [], [], []
        for alloc in nc.m.functions[0].allocations:
            if not isinstance(alloc, mybir.MemoryLocationSet):
                continue
            name = alloc.memorylocations[0].name
            if alloc.kind == "ExternalInput":
                if name != partition_name:
                    in_names.append(name)
            elif alloc.kind == "ExternalOutput":
                shape = tuple(alloc.tensor_shape)
                dtype = mybir.dt.np(alloc.dtype)
                out_names.append(name)
                out_avals.append(jax.core.ShapedArray(shape, dtype))
                zero_outs.append(np.zeros(shape, dtype))
        n_params = len(in_names)
        all_in_names = list(in_names) + list(out_names)
        if partition_name is not None:
            all_in_names.append(partition_name)
        self.in_names, self.out_names = in_names, out_names
        self.out_avals, self.zero_outs = out_avals, zero_outs

        def _body(*args):
            operands = list(args)
            if partition_name is not None:
                operands.append(partition_id_tensor())
            outs = _bass_exec_p.bind(
                *operands,
                out_avals=tuple(out_avals),
                in_names=tuple(all_in_names),
                out_names=tuple(out_names),
                lowering_input_output_aliases=(),
                sim_require_finite=True,
                sim_require_nnan=True,
                nc=nc,
            )
            return tuple(outs)

        devices = jax.devices()[:n_cores]
        assert len(devices) == n_cores, (
            f"need {n_cores} neuron cores, found {len(jax.devices())}"
        )
        mesh = Mesh(np.asarray(devices), ("core",))
        n_outs = len(out_avals)
        self.sharding = jax.sharding.NamedSharding(mesh, PartitionSpec("core"))
        self.fn = jax.jit(
            shard_map(
                _body,
                mesh=mesh,
                in_specs=(PartitionSpec("core"),) * (n_params + n_outs),
                out_specs=(PartitionSpec("core"),) * n_outs,
                check_rep=False,
            ),
            keep_unused=True,
        )

    def run(self, in_maps):
        n = self.n_cores
        concat_in = [
            np.concatenate(
                [np.asarray(in_maps[c][name]) for c in range(n)], axis=0
            )
            for name in self.in_names
        ]
        concat_zero = [
            np.zeros((n * z.shape[0], *z.shape[1:]), z.dtype)
            for z in self.zero_outs
        ]
        out_arrs = self.fn(*concat_in, *concat_zero)
        jax.block_until_ready(out_arrs)
        return [
            {
                name: np.asarray(out_arrs[i]).reshape(
                    n, *self.out_avals[i].shape
                )[c]
                for i, name in enumerate(self.out_names)
            }
            for c in range(n)
        ]


_RUNNER_CACHE = {}


def _get_runner(reps=1):
    key = reps
    if key not in _RUNNER_CACHE:
        nc = build_core_kernel(T=2048, C=2048, HL=8, reps=reps, pool_mode="queue")
        _RUNNER_CACHE[key] = _SpmdRunner(nc, 8)
    return _RUNNER_CACHE[key]


def kernel(x, Wq, bq, Wk, bk, Wv, bv, Wo, bo, _reps=1):
    """Causal self-attention with RoPE. Full inputs in, full output out.

    Shards batch (4) x head-halves (2) across the 8 NeuronCores; each
    core computes a partial [T, C] output; core pairs are summed on the
    host (the tensor-parallel all-reduce) and bo is added.
    """
    x = np.ascontiguousarray(np.asarray(x, dtype=np.float32))
    B, T, C = x.shape
    in_maps = host_prepare(
        np.asarray(x), np.asarray(Wq), np.asarray(bq), np.asarray(Wk),
        np.asarray(bk), np.asarray(Wv), np.asarray(bv), np.asarray(Wo),
        np.asarray(bo),
    )
    runner = _get_runner(_reps)
    results = runner.run(in_maps)
    return assemble(results, np.asarray(bo, dtype=np.float32), B, T, C)



# revision 2
# speedup vs baseline: 3.9723x; 3.9723x over previous
"""Causal self-attention with RoPE for trn2, 8-core SPMD.

Sharding: core i handles batch b = i//2 and heads [8*(i%2), 8*(i%2)+8).
Each core computes a partial output [T, C] (bf16); host sums core pairs
in fp32 and adds bo.

Per-core schedule (all matmul inputs bf16 — fp32r runs far below peak on
this hardware, 16-bit runs at ~1 cyc/row):
  - q/k/v/y stay SBUF-resident; no DRAM round trips.
  - P1v: v = x@Wv+bv for all local heads, [t,vch] layout.
  - per head, software-pipelined: projections for head h+1 are emitted
    before attention of head h so the in-order PE never starves.
  - RoPE: ACT evicts PSUM with bias folded in; two DVE multiplies with
    host-prepared cos/sin tables (de-interleaved layout via host column
    permutation of Wq/Wk), half-partition swap via SBUF-SBUF DMA, DVE add.
  - attention per (head, 512-wide q-slice): St[tk,tq] = k_blk^T q in
    paired PSUM banks [128,1024]; causal diagonal handled by an additive
    -1e6 mask folded into the PE accumulation (identity-matmul trick) so
    exp needs no separate masking pass; causal trimming skips fully
    masked columns in St/exp/y.
  - softmax denominator: E tiles accumulated on DVE in fp16 (two chains),
    one ones-matmul pair per q-slice; normalization = DVE reciprocal +
    multiply folded into the y eviction.
  - y-matmuls lag the St/exp pipeline by 2 pairs to hide ACT latency.
  - P3: out[t,c] = sum_h y_h^T Wo rows, bf16 partials DMA'd out.
"""
from contextlib import ExitStack

import numpy as np

import concourse.bacc as bacc
import concourse.tile as tile
from concourse import mybir

F32 = mybir.dt.float32
F16 = mybir.dt.bfloat16
FP16 = mybir.dt.float16
AF = mybir.ActivationFunctionType
ALU = mybir.AluOpType

N_HEAD = 16
HEAD_DIM = 128
ROPE_BASE = 10000.0


def build_core_kernel(T=2048, C=2048, HL=8, reps=1, pool_mode="queue"):
    """One core's program: full T, C channels, HL local heads."""
    D = HEAD_DIM
    CL = HL * D            # local q/k/v channels (1024)
    NCT = C // 128         # c-tiles (16)
    NQ = T // 512          # 512-wide t slices (4)
    NT = T // 128          # 128-wide t tiles (16)
    NCQ = C // 512         # out-proj column groups (4)
    scale = 1.0 / float(np.sqrt(D))

    nc = bacc.Bacc("TRN2", target_bir_lowering=False, debug=False)

    xT_d = nc.dram_tensor("xT", [C, T], F16, kind="ExternalInput")
    wq_d = nc.dram_tensor("wq", [HL, 128, NCT, 128], F16, kind="ExternalInput")
    wk_d = nc.dram_tensor("wk", [HL, 128, NCT, 128], F16, kind="ExternalInput")
    wv_d = nc.dram_tensor("wv", [2, 128, NCT, 512], F16, kind="ExternalInput")
    wo_d = nc.dram_tensor("wo", [NCQ, 128, HL, 512], F16, kind="ExternalInput")
    bq_d = nc.dram_tensor("bq", [CL], F32, kind="ExternalInput")
    bk_d = nc.dram_tensor("bk", [CL], F32, kind="ExternalInput")
    bv_d = nc.dram_tensor("bv", [CL], F16, kind="ExternalInput")
    cos_d = nc.dram_tensor("cos2", [128, T], F16, kind="ExternalInput")
    sin_d = nc.dram_tensor("sin2", [128, T], F16, kind="ExternalInput")
    mneg_d = nc.dram_tensor("mneg", [128, 512], F16, kind="ExternalInput")
    iden_d = nc.dram_tensor("iden", [128, 128], F16, kind="ExternalInput")
    ones_d = nc.dram_tensor("ones", [128, 128], FP16, kind="ExternalInput")
    out_d = nc.dram_tensor("out_p", [T, C], F16, kind="ExternalOutput")

    with tile.TileContext(nc, pool_alloc_mode=pool_mode) as tc, ExitStack() as top:
        # --- PSUM: 8 banks exactly ---
        psS = top.enter_context(tc.tile_pool(name="psS", bufs=2, space="PSUM"))
        psY = top.enter_context(tc.tile_pool(name="psY", bufs=1, space="PSUM"))
        psD = top.enter_context(tc.tile_pool(name="psD", bufs=1, space="PSUM"))
        psP = top.enter_context(tc.tile_pool(name="psP", bufs=2, space="PSUM"))

        const = top.enter_context(tc.tile_pool(name="const", bufs=1))
        ones_sb = const.tile([128, 128], FP16)
        mneg_sb = const.tile([128, 512], F16)
        iden_sb = const.tile([128, 128], F16)
        cos_sb = const.tile([128, T], F16)
        sin_sb = const.tile([128, T], F16)
        bq_sb = const.tile([128, HL], F32)
        bk_sb = const.tile([128, HL], F32)
        nc.sync.dma_start(out=ones_sb, in_=ones_d[:, :])
        nc.sync.dma_start(out=mneg_sb, in_=mneg_d[:, :])
        nc.sync.dma_start(out=iden_sb, in_=iden_d[:, :])
        nc.sync.dma_start(out=cos_sb, in_=cos_d[:, :])
        nc.sync.dma_start(out=sin_sb, in_=sin_d[:, :])
        nc.sync.dma_start(out=bq_sb, in_=bq_d.rearrange("(h p) -> p h", p=128))
        nc.sync.dma_start(out=bk_sb, in_=bk_d.rearrange("(h p) -> p h", p=128))

        # resident tensors
        res = top.enter_context(tc.tile_pool(name="res", bufs=1))
        v_sb = res.tile([128, NT, CL], F16)      # [t-local, t-tile, v-chan]
        y_sb = res.tile([128, HL, T], F16)       # [d, head, t]

        rep_ctx = tc.For_i(0, reps, 1) if reps > 1 else None
        if rep_ctx is not None:
            top.enter_context(rep_ctx)

        with ExitStack() as body:
            xp = body.enter_context(tc.tile_pool(name="xp", bufs=1))
            xT_sb = xp.tile([128, NCT, T], F16)
            for ct in range(NCT):
                nc.sync.dma_start(
                    out=xT_sb[:, ct, :],
                    in_=xT_d[ct * 128 : (ct + 1) * 128, :],
                )

            # ---------------- P1v: v projection (all heads) ----------------
            with ExitStack() as p1v:
                wvp = p1v.enter_context(tc.tile_pool(name="wvp", bufs=1))
                bvp = p1v.enter_context(tc.tile_pool(name="bvp", bufs=1))
                bv_sb = bvp.tile([128, CL], F16)
                nc.sync.dma_start(out=bv_sb, in_=bv_d[:].partition_broadcast(128))
                for g in range(2):
                    gs = slice(g * 512, (g + 1) * 512)
                    wv_sb = wvp.tile([128, NCT, 512], F16, tag="wv")
                    nc.sync.dma_start(out=wv_sb, in_=wv_d[g])
                    for tt in range(NT):
                        xl = xT_sb[:, :, tt * 128 : (tt + 1) * 128]
                        ps = psP.tile([128, 512], F32, tag="mm")
                        for ct in range(NCT):
                            nc.tensor.matmul(
                                ps[:],
                                xl[:, ct, :],
                                wv_sb[:, ct, :],
                                start=(ct == 0),
                                stop=(ct == NCT - 1),
                            )
                        nc.vector.tensor_tensor(
                            v_sb[:, tt, gs], ps[:], bv_sb[:, gs], op=ALU.add
                        )

            # ---------------- per-head: P1qk(h+1) then P2(h) ----------------
            with ExitStack() as ph:
                wqk = ph.enter_context(tc.tile_pool(name="wqk", bufs=2))
                qkp = ph.enter_context(tc.tile_pool(name="qkp", bufs=2))
                ev = ph.enter_context(tc.tile_pool(name="ev", bufs=2))
                ep = ph.enter_context(tc.tile_pool(name="ep", bufs=5))
                ap_pool = ph.enter_context(tc.tile_pool(name="accp", bufs=2))
                rp = ph.enter_context(tc.tile_pool(name="rp", bufs=2))

                qk_tiles = {}

                def emit_p1qk(h):
                    q_sb = qkp.tile([128, T], F16, tag="qh")
                    k_sb = qkp.tile([128, T], F16, tag="kh")
                    qk_tiles[h] = (q_sb, k_sb)
                    for w_d, b_sb, o_sb, wtag in (
                        (wk_d, bk_sb, k_sb, "wks"),
                        (wq_d, bq_sb, q_sb, "wqs"),
                    ):
                        w_sb = wqk.tile([128, NCT, 128], F16, tag=wtag)
                        nc.sync.dma_start(out=w_sb, in_=w_d[h])
                        for s in range(NQ):
                            ts = slice(s * 512, (s + 1) * 512)
                            ps = psP.tile([128, 512], F32, tag="mm")
                            for ct in range(NCT):
                                nc.tensor.matmul(
                                    ps[:],
                                    w_sb[:, ct, :],
                                    xT_sb[:, ct, ts],
                                    start=(ct == 0),
                                    stop=(ct == NCT - 1),
                                )
                            raw = ev.tile([128, 512], F16, tag="qkraw")
                            nc.scalar.activation(
                                out=raw[:], in_=ps[:], func=AF.Identity,
                                bias=b_sb[:, h : h + 1], scale=1.0,
                            )
                            p1t = ev.tile([128, 512], F16, tag="p1t")
                            p2t = ev.tile([128, 512], F16, tag="p2t")
                            nc.vector.tensor_tensor(
                                p1t[:], raw[:], cos_sb[:, ts], op=ALU.mult
                            )
                            nc.vector.tensor_tensor(
                                p2t[:], raw[:], sin_sb[:, ts], op=ALU.mult
                            )
                            swp = ev.tile([128, 512], F16, tag="swp")
                            nc.sync.dma_start(out=swp[0:64, :], in_=p2t[64:128, :])
                            nc.sync.dma_start(out=swp[64:128, :], in_=p2t[0:64, :])
                            nc.vector.tensor_tensor(
                                o_sb[:, ts], p1t[:], swp[:], op=ALU.add
                            )

                Y_LAG = 2

                def emit_p2(h):
                    q_sb, k_sb = qk_tiles.pop(h)
                    hs = slice(h * 128, (h + 1) * 128)
                    for j in range(NQ):
                        nblk = 4 * (j + 1)
                        npair = nblk // 2
                        psd = psD.tile([128, 512], F32)
                        psy = psY.tile([128, 512], F32)
                        acc0 = ap_pool.tile([128, 512], FP16, tag="acc0")
                        acc1 = ap_pool.tile([128, 512], FP16, tag="acc1")
                        acc_off = [None, None]
                        epairs = []

                        def emit_y(i):
                            et_i, offs_i = epairs[i]
                            for half in range(2):
                                b = 2 * i + half
                                off = offs_i[half]
                                sl = slice(half * 512 + off, (half + 1) * 512)
                                nc.tensor.matmul(
                                    psy[:, off:512],
                                    v_sb[:, b, hs],
                                    et_i[:, sl],
                                    start=(b == 0),
                                    stop=(b == nblk - 1),
                                )

                        for i in range(npair):
                            ps2 = psS.tile([128, 1024], F32, tag="st")
                            offs = []
                            for half in range(2):
                                b = 2 * i + half
                                off = max(0, (b - 4 * j) * 128)
                                offs.append(off)
                                diag = b >= 4 * j
                                if diag:
                                    nc.tensor.matmul(
                                        ps2[:, half * 512 + off : (half + 1) * 512],
                                        iden_sb[:],
                                        mneg_sb[:, 0 : 512 - off],
                                        start=True,
                                        stop=False,
                                    )
                                nc.tensor.matmul(
                                    ps2[:, half * 512 + off : (half + 1) * 512],
                                    k_sb[:, b * 128 : (b + 1) * 128],
                                    q_sb[:, j * 512 + off : (j + 1) * 512],
                                    start=not diag,
                                    stop=True,
                                )
                            et = ep.tile([128, 1024], F16, tag="e")
                            if offs[0] == 0 and offs[1] == 0:
                                nc.scalar.activation(
                                    out=et[:], in_=ps2[:], func=AF.Exp, scale=scale
                                )
                            else:
                                for half in range(2):
                                    sl = slice(
                                        half * 512 + offs[half], (half + 1) * 512
                                    )
                                    nc.scalar.activation(
                                        out=et[:, sl], in_=ps2[:, sl],
                                        func=AF.Exp, scale=scale,
                                    )
                            # denominator accumulation on DVE (two chains)
                            for half, acc in ((0, acc0), (1, acc1)):
                                off = offs[half]
                                sl = slice(half * 512 + off, (half + 1) * 512)
                                if acc_off[half] is None:
                                    acc_off[half] = off
                                    nc.vector.tensor_copy(
                                        out=acc[:, off:512], in_=et[:, sl]
                                    )
                                else:
                                    nc.vector.tensor_tensor(
                                        acc[:, off:512], acc[:, off:512],
                                        et[:, sl], op=ALU.add,
                                    )
                            epairs.append((et, offs))
                            if i >= Y_LAG:
                                emit_y(i - Y_LAG)
                        for i in range(max(0, npair - Y_LAG), npair):
                            emit_y(i)

                        nc.tensor.matmul(
                            psd[:, acc_off[0] : 512],
                            ones_sb[:], acc0[:, acc_off[0] : 512],
                            start=True, stop=False,
                        )
                        nc.tensor.matmul(
                            psd[:, acc_off[1] : 512],
                            ones_sb[:], acc1[:, acc_off[1] : 512],
                            start=False, stop=True,
                        )
                        recb = rp.tile([128, 512], F32, tag="recb")
                        nc.vector.reciprocal(out=recb[:], in_=psd[:])
                        nc.vector.tensor_tensor(
                            y_sb[:, h, j * 512 : (j + 1) * 512],
                            psy[:], recb[:], op=ALU.mult,
                        )

                emit_p1qk(0)
                for h in range(HL):
                    if h + 1 < HL:
                        emit_p1qk(h + 1)
                    emit_p2(h)

        # ---------------- P3: output projection ----------------
        with ExitStack() as p3:
            wop = p3.enter_context(tc.tile_pool(name="wop", bufs=2))
            op = p3.enter_context(tc.tile_pool(name="op", bufs=3))
            for cq in range(NCQ):
                wo_sb = wop.tile([128, HL, 512], F16, tag="wo")
                nc.sync.dma_start(out=wo_sb, in_=wo_d[cq])
                for tt in range(NT):
                    tsl = slice(tt * 128, (tt + 1) * 128)
                    ps = psP.tile([128, 512], F32, tag="mm")
                    for h in range(HL):
                        nc.tensor.matmul(
                            ps[:],
                            y_sb[:, h, tsl],
                            wo_sb[:, h, :],
                            start=(h == 0),
                            stop=(h == HL - 1),
                        )
                    ot = op.tile([128, 512], F16)
                    nc.scalar.copy(out=ot[:], in_=ps[:])
                    nc.sync.dma_start(
                        out=out_d[tsl, cq * 512 : (cq + 1) * 512], in_=ot[:]
                    )

    nc.finalize()
    return nc


def _col_perm(CL):
    """Per-head even/odd de-interleave of columns."""
    perm = []
    for h in range(CL // 128):
        base = h * 128
        perm += [base + i for i in range(0, 128, 2)]
        perm += [base + i for i in range(1, 128, 2)]
    return np.array(perm)


def host_prepare(x, Wq, bq, Wk, bk, Wv, bv, Wo, bo, T=None):
    """Build the 8 per-core input maps. x: [B, T, C] fp32."""
    B, Tfull, C = x.shape
    if T is None:
        T = Tfull
    D = HEAD_DIM
    import ml_dtypes
    F16 = ml_dtypes.bfloat16
    perm = _col_perm(C)
    Wq_p = np.ascontiguousarray(Wq[:, perm])
    Wk_p = np.ascontiguousarray(Wk[:, perm])
    bq_p = np.ascontiguousarray(bq[perm])
    bk_p = np.ascontiguousarray(bk[perm])

    # rope tables (fp32 math, stored fp16)
    inv = (1.0 / (ROPE_BASE ** (np.arange(0, D, 2, dtype=np.float32) / D))).astype(
        np.float32
    )
    pos = np.arange(T, dtype=np.float32)
    th = pos[None, :] * inv[:, None]          # [64, T]
    cos1 = np.cos(th).astype(np.float32)
    sin1 = np.sin(th).astype(np.float32)
    cos2 = np.concatenate([cos1, cos1], axis=0).astype(F16)
    sin2 = np.concatenate([sin1, -sin1], axis=0).astype(F16)

    p = np.arange(128)[:, None]
    f = np.arange(512)[None, :]
    mneg = np.where(p > f, -1.0e6, 0.0).astype(F16)
    iden = np.eye(128).astype(F16)

    NCT = C // 128

    def tile_qk(W):  # [C, 1024] -> [8, 128, NCT, 128]
        return np.ascontiguousarray(
            W.reshape(NCT, 128, 8, 128).transpose(2, 1, 0, 3).astype(F16)
        )

    def tile_v(W):  # [C, 1024] -> [2, 128, NCT, 512]
        return np.ascontiguousarray(
            W.reshape(NCT, 128, 2, 512).transpose(2, 1, 0, 3).astype(F16)
        )

    def tile_o(W):  # [1024, C] -> [C//512, 128, 8, 512]
        return np.ascontiguousarray(
            W.reshape(8, 128, C // 512, 512).transpose(2, 1, 0, 3).astype(F16)
        )

    in_maps = []
    for core in range(8):
        b, half = core // 2, core % 2
        cl = slice(half * 1024, (half + 1) * 1024)
        xT = np.ascontiguousarray(x[b, :T].T.astype(F16))
        in_maps.append(
            {
                "xT": xT,
                "wq": tile_qk(Wq_p[:, cl]),
                "wk": tile_qk(Wk_p[:, cl]),
                "wv": tile_v(Wv[:, cl]),
                "wo": tile_o(Wo[cl.start : cl.stop, :]),
                "bq": np.ascontiguousarray(bq_p[cl]).astype(np.float32),
                "bk": np.ascontiguousarray(bk_p[cl]).astype(np.float32),
                "bv": np.ascontiguousarray(bv[cl]).astype(F16),
                "cos2": cos2,
                "sin2": sin2,
                "mneg": mneg,
                "iden": iden,
                "ones": np.ones((128, 128), dtype=np.float16),
            }
        )
    return in_maps


def assemble(results, bo, B, T, C):
    out = np.empty((B, T, C), dtype=np.float32)
    bo32 = np.asarray(bo, dtype=np.float32)
    for b in range(B):
        out[b] = (
            results[2 * b]["out_p"].astype(np.float32)
            + results[2 * b + 1]["out_p"].astype(np.float32)
            + bo32[None, :]
        )
    return out


# ---------------------------------------------------------------------------
# SPMD execution via PJRT/axon (compiles once per process, reusable)
# ---------------------------------------------------------------------------
import jax
from jax.sharding import Mesh, PartitionSpec
from jax.experimental.shard_map import shard_map

from concourse.bass2jax import (
    _bass_exec_p,
    install_neuronx_cc_hook,
    partition_id_tensor,
)


class _SpmdRunner:
    def __init__(self, nc, n_cores):
        install_neuronx_cc_hook()
        self.nc = nc
        self.n_cores = n_cores
        partition_name = (
            nc.partition_id_tensor.name if nc.partition_id_tensor else None
        )
        in_names, out_names, out_avals, zero_outs = [], [], [], []
        for alloc in nc.m.functions[0].allocations:
            if not isinstance(alloc, mybir.MemoryLocationSet):
                continue
            name = alloc.memorylocations[0].name
            if alloc.kind == "ExternalInput":
                if name != partition_name:
                    in_names.append(name)
            elif alloc.kind == "ExternalOutput":
                shape = tuple(alloc.tensor_shape)
                dtype = mybir.dt.np(alloc.dtype)
                out_names.append(name)
                out_avals.append(jax.core.ShapedArray(shape, dtype))
                zero_outs.append(np.zeros(shape, dtype))
        n_params = len(in_names)
        all_in_names = list(in_names) + list(out_names)
        if partition_name is not None:
            all_in_names.append(partition_name)
        self.in_names, self.out_names = in_names, out_names
        self.out_avals, self.zero_outs = out_avals, zero_outs

        def _body(*args):
            operands = list(args)
            if partition_name is not None:
                operands.append(partition_id_tensor())
            outs = _bass_exec_p.bind(
                *operands,
                out_avals=tuple(out_avals),
                in_names=tuple(all_in_names),
                out_names=tuple(out_names),
                lowering_input_output_aliases=(),
                sim_require_finite=True,
                sim_require_nnan=True,
                nc=nc,
            )
            return tuple(outs)

        devices = jax.devices()[:n_cores]
        assert len(devices) == n_cores, (
            f"need {n_cores} neuron cores, found {len(jax.devices())}"
        )
        mesh = Mesh(np.asarray(devices), ("core",))
        n_outs = len(out_avals)
        self.sharding = jax.sharding.NamedSharding(mesh, PartitionSpec("core"))
        self.fn = jax.jit(
            shard_map(
                _body,
                mesh=mesh,
                in_specs=(PartitionSpec("core"),) * (n_params + n_outs),
                out_specs=(PartitionSpec("core"),) * n_outs,
                check_rep=False,
            ),
            keep_unused=True,
        )

    def run(self, in_maps):
        n = self.n_cores
        concat_in = [
            np.concatenate(
                [np.asarray(in_maps[c][name]) for c in range(n)], axis=0
            )
            for name in self.in_names
        ]
        concat_zero = [
            np.zeros((n * z.shape[0], *z.shape[1:]), z.dtype)
            for z in self.zero_outs
        ]
        out_arrs = self.fn(*concat_in, *concat_zero)
        jax.block_until_ready(out_arrs)
        return [
            {
                name: np.asarray(out_arrs[i]).reshape(
                    n, *self.out_avals[i].shape
                )[c]
                for i, name in enumerate(self.out_names)
            }
            for c in range(n)
        ]


_RUNNER_CACHE = {}


def _get_runner(reps=1):
    key = reps
    if key not in _RUNNER_CACHE:
        nc = build_core_kernel(T=2048, C=2048, HL=8, reps=reps, pool_mode="queue")
        _RUNNER_CACHE[key] = _SpmdRunner(nc, 8)
    return _RUNNER_CACHE[key]


def kernel(x, Wq, bq, Wk, bk, Wv, bv, Wo, bo, _reps=1):
    """Causal self-attention with RoPE. Full inputs in, full output out."""
    x = np.ascontiguousarray(np.asarray(x, dtype=np.float32))
    B, T, C = x.shape
    in_maps = host_prepare(
        np.asarray(x), np.asarray(Wq), np.asarray(bq), np.asarray(Wk),
        np.asarray(bk), np.asarray(Wv), np.asarray(bv), np.asarray(Wo),
        np.asarray(bo),
    )
    runner = _get_runner(_reps)
    results = runner.run(in_maps)
    return assemble(results, np.asarray(bo, dtype=np.float32), B, T, C)
